# revision 6
# baseline (speedup 1.0000x reference)
"""Multi-head attention (B=4, N=2048, D=512, H=8, Dh=64) on 8 trn2 cores.

Sharding: core c handles batch b = c//2 and head-group hg = c%2 (4 heads =
2 pairs).  Each core computes its batch's attention output for its 4 heads
plus the partial output projection (w_out rows for those heads); the host
sums the two head-group partials per batch.

v2 layout (all PE operands bf16, f32 PSUM accumulation):
 - x arrives transposed ([D, N]) so every contraction runs over partitions.
 - Q^T/K^T tiles [128, N]: rows 0:64 even head of a pair, 64:128 odd head.
 - Scores S^T per pair computed as row-tiled matmul pairs (tile rows 0/64
   stream the two heads concurrently through the PE array).
 - exp on the scalar engine straight out of PSUM ([128, 1536] slices).
 - PV col-tiled: even head [V|ones] at array cols 0:65 (O even + den0 at
   psum partition 64), a [ones,0..] tile at cols 96:128 gives den1 at
   partition 96, odd head V at cols 64:128 puts O_odd at psum partitions
   64:128 so the pair's O^T stacks into one [128, N] sbuf tile.
 - Softmax denominators are batch-reciprocaled ([16,128] DVE op per
   i-block) and broadcast back via a DRAM bounce.
 - Output projection contracts the stacked pair tiles (contraction 128).
"""

import sys

for p in ("/opt/trn_rl_repo", "/root/.axon_site/_ro/trn_rl_repo"):
    if p not in sys.path:
        sys.path.insert(0, p)

from contextlib import ExitStack

import numpy as np
import ml_dtypes

import concourse.bass as bass
import concourse.mybir as mybir
import concourse.tile as tile
from concourse import bacc
from concourse.bass_utils import run_bass_kernel_spmd

F32 = mybir.dt.float32
BF16 = mybir.dt.bfloat16
AF = mybir.ActivationFunctionType
BF16NP = ml_dtypes.bfloat16

N_CORES = 8
B, N, D = 4, 2048, 512
HEADS = 8
DH = 64
SCALE = DH**-0.5
HPC = 4  # heads per core (2 pairs)
P = 128
NDT = D // P  # 4 d-tiles
NJT = N // P  # 16 j-tiles
IB = 512  # i-block
NIB = N // IB  # 4 i-blocks
UNIT = 3  # score psum slices ([128,512]) per exp instruction
NSL = 2 * NJT  # score slices per (ib, pair): (jt, par)

N_REPS = 1  # replications of the whole body inside one NEFF (for timing)


def build_program(n_reps: int = N_REPS):
    nc = bacc.Bacc("TRN2", target_bir_lowering=False, debug=False,
                   num_devices=N_CORES)
    xT = nc.dram_tensor("xT", [D, N], BF16, kind="ExternalInput").ap()
    wqk = nc.dram_tensor("wqk", [D, 2 * HPC * DH], BF16, kind="ExternalInput").ap()
    wv = nc.dram_tensor("wv", [D, HPC * DH], BF16, kind="ExternalInput").ap()
    wo = nc.dram_tensor("wo", [HPC * DH, D], BF16, kind="ExternalInput").ap()
    bias = nc.dram_tensor("bias", [D, 1], F32, kind="ExternalInput").ap()
    yT = nc.dram_tensor("yT", [D, N], F32, kind="ExternalOutput").ap()
    # DRAM scratch: raw denominators and their reciprocals, per i-block.
    # den_d[ib, par, pair*IB:+IB]; rden_d likewise.
    den_d = nc.dram_tensor("den_d", [NIB, 2, 2 * IB], F32).ap()
    rden_d = nc.dram_tensor("rden_d", [NIB, 2, 2 * IB], F32).ap()

    with tile.TileContext(nc) as tc, ExitStack() as ctx:
        sb = ctx.enter_context(tc.tile_pool(name="sb", bufs=1))
        if n_reps > 1:
            ctx.enter_context(tc.For_i(0, n_reps, 1))

        for _rep in range(1):
            # preload the exp activation table while the input DMAs run
            warm = sb.tile([1, 16], F32, tag="warm", bufs=1)
            nc.vector.memset(warm, 0.0)
            nc.scalar.activation(warm, warm, AF.Exp, scale=1.0)

            # ---------------- phase 1: load + QKV projection ----------------
            wqk_sb = []
            wv_sb = []
            bias_sb = []
            for dt in range(NDT):
                t = sb.tile([P, 2 * HPC * DH], BF16, tag="wqk", bufs=NDT)
                nc.sync.dma_start(out=t, in_=wqk[dt * P:(dt + 1) * P, :])
                wqk_sb.append(t)
                t = sb.tile([P, HPC * DH], BF16, tag="wv", bufs=NDT)
                nc.sync.dma_start(out=t, in_=wv[dt * P:(dt + 1) * P, :])
                wv_sb.append(t)
                t = sb.tile([P, 1], F32, tag="bias", bufs=NDT)
                nc.sync.dma_start(out=t, in_=bias[dt * P:(dt + 1) * P, :])
                bias_sb.append(t)
            wo_sb = []
            for pr in range(2):
                t = sb.tile([P, D], BF16, tag="wo", bufs=2)
                nc.sync.dma_start(out=t, in_=wo[pr * P:(pr + 1) * P, :])
                wo_sb.append(t)

            xt_sb = []
            for dt in range(NDT):
                t = sb.tile([P, N], BF16, tag="xt", bufs=NDT)
                nc.sync.dma_start(out=t, in_=xT[dt * P:(dt + 1) * P, :])
                xt_sb.append(t)

            # ones stationary for the den1 column tile: col 0 = 1, rest 0
            ones32 = sb.tile([P, 32], BF16, tag="ones", bufs=1)
            nc.vector.memset(ones32, 0.0)
            nc.vector.memset(ones32.rearrange("p (a c) -> p a c", c=1)[:, 0, :], 1.0)

            # Q^T/K^T tiles [128, N]; rows 0:64 even head of pair, 64:128 odd.
            # et: 0 = Q pair0, 1 = Q pair1, 2 = K pair0, 3 = K pair1
            qkt_sb = []
            with tc.tile_pool(name="ps1", bufs=1, space="PSUM") as ps1:
                for et in range(4):
                    t = sb.tile([P, N], BF16, tag="qkt", bufs=4)
                    qkt_sb.append(t)
                    for nb in range(NIB):
                        pq = ps1.tile([P, IB], F32, tag="qk", bufs=4)
                        for dt in range(NDT):
                            nc.tensor.matmul(
                                pq,
                                lhsT=wqk_sb[dt][:, et * P:(et + 1) * P],
                                rhs=xt_sb[dt][:, nb * IB:(nb + 1) * IB],
                                start=(dt == 0), stop=(dt == NDT - 1),
                            )
                        nc.vector.tensor_copy(t[:, nb * IB:(nb + 1) * IB], pq)

                # V natural [n, e] with a ones column per head: [128, 4*65]
                v_sb = []
                for nt in range(NJT):
                    t = sb.tile([P, HPC * (DH + 1)], BF16, tag="v", bufs=NJT)
                    v_sb.append(t)
                    pv = ps1.tile([P, HPC * DH], F32, tag="v", bufs=2)
                    for dt in range(NDT):
                        nc.tensor.matmul(
                            pv,
                            lhsT=xt_sb[dt][:, nt * P:(nt + 1) * P],
                            rhs=wv_sb[dt],
                            start=(dt == 0), stop=(dt == NDT - 1),
                        )
                    nc.vector.tensor_copy(
                        t.rearrange("p (h c) -> p h c", c=DH + 1)[:, :, 0:DH],
                        pv.rearrange("p (h c) -> p h c", c=DH),
                    )
                    nc.vector.memset(
                        t.rearrange("p (h c) -> p h c", c=DH + 1)[:, :, DH:DH + 1],
                        1.0,
                    )

            # ---------------- phase 2+3: attention + projection, per i-block --
            # O^T stacked per pair: [128, N] (rows 0:64 even head, 64:128 odd),
            # unnormalized until the den-reciprocal multiply.
            ot_sb = [sb.tile([P, N], BF16, tag="ot", bufs=2, name=f"ot{_p}")
                     for _p in range(2)]

            slices = [(jt, par) for jt in range(NJT) for par in range(2)]
            units = [slices[i:i + UNIT] for i in range(0, len(slices), UNIT)]

            with tc.tile_pool(name="ps2", bufs=1, space="PSUM") as ps2:
                for ib in range(NIB):
                    den_c = sb.tile([P, 2 * IB], F32, tag="den", bufs=2)
                    for pair in range(2):
                        # pv_a bank: even-head O at partitions 0:65 (incl den0
                        # at 64), den1 at partition 96. pv_b bank: odd-head O
                        # at partitions 64:128.
                        pv_a = ps2.tile([P, IB], F32, tag="pv", bufs=2)
                        pv_b = ps2.tile([P, IB], F32, tag="pv", bufs=2)
                        for unit in units:
                            su = ps2.tile([P, len(unit) * IB], F32, tag="s",
                                          bufs=2)
                            for k, (jt, par) in enumerate(unit):
                                nc.tensor.matmul(
                                    su[:, k * IB:(k + 1) * IB],
                                    lhsT=qkt_sb[2 + pair][
                                        par * DH:(par + 1) * DH,
                                        jt * P:(jt + 1) * P],
                                    rhs=qkt_sb[pair][
                                        par * DH:(par + 1) * DH,
                                        ib * IB:(ib + 1) * IB],
                                    start=True, stop=True,
                                )
                            es = sb.tile([P, len(unit) * IB], BF16, tag="big",
                                         bufs=8)
                            nc.scalar.activation(es, su, AF.Exp, scale=SCALE)
                            for k, (jt, par) in enumerate(unit):
                                rhs = es[:, k * IB:(k + 1) * IB]
                                st = jt == 0
                                sp = jt == NJT - 1
                                he = 2 * pair
                                ho = 2 * pair + 1
                                if par == 0:
                                    # even head [V|ones] at cols 0:65
                                    nc.tensor.matmul(
                                        pv_a[0:DH + 1, :],
                                        lhsT=v_sb[jt][:, he * (DH + 1):
                                                      (he + 1) * (DH + 1)],
                                        rhs=rhs, start=st, stop=sp,
                                        skip_group_check=True,
                                    )
                                else:
                                    # den for odd head at col/partition 96
                                    nc.tensor.matmul(
                                        pv_a[96:128, :],
                                        lhsT=ones32,
                                        rhs=rhs, start=st, stop=sp,
                                        tile_position=(0, 96),
                                        skip_group_check=True,
                                    )
                                    # odd head V at cols 64:128
                                    nc.tensor.matmul(
                                        pv_b[DH:128, :],
                                        lhsT=v_sb[jt][:, ho * (DH + 1):
                                                      ho * (DH + 1) + DH],
                                        rhs=rhs, start=st, stop=sp,
                                        skip_group_check=True,
                                    )
                        # drain: O tiles to sbuf (partition-aligned), dens to
                        # the collect tile rows 64 (par0) / 96 (par1).
                        nc.vector.tensor_copy(
                            ot_sb[pair][0:DH, ib * IB:(ib + 1) * IB],
                            pv_a[0:DH, :],
                        )
                        nc.vector.tensor_copy(
                            ot_sb[pair][DH:128, ib * IB:(ib + 1) * IB],
                            pv_b[DH:128, :],
                        )
                        nc.vector.tensor_copy(
                            den_c[DH:DH + 1, pair * IB:(pair + 1) * IB],
                            pv_a[DH:DH + 1, :],
                        )
                        nc.vector.tensor_copy(
                            den_c[96:97, pair * IB:(pair + 1) * IB],
                            pv_a[96:97, :],
                        )
                    # batch-reciprocal this i-block's denominators via DRAM
                    nc.sync.dma_start(out=den_d[ib, 0:1, :],
                                      in_=den_c[DH:DH + 1, :])
                    nc.sync.dma_start(out=den_d[ib, 1:2, :],
                                      in_=den_c[96:97, :])
                    rc = sb.tile([16, P], F32, tag="rc", bufs=2)
                    nc.sync.dma_start(
                        out=rc,
                        in_=den_d[ib].rearrange("a (q p) -> (a q) p", p=P))
                    nc.vector.reciprocal(rc, rc)
                    nc.sync.dma_start(
                        out=rden_d[ib].rearrange("a (q p) -> (a q) p", p=P),
                        in_=rc)
                    for pair in range(2):
                        rb = sb.tile([P, IB], F32, tag="rb", bufs=4)
                        for par in range(2):
                            src = rden_d[ib, par:par + 1,
                                         pair * IB:(pair + 1) * IB]
                            bcast = bass.AP(
                                tensor=src.tensor, offset=src.offset,
                                ap=[[0, DH]] + [list(d) for d in src.ap[-1:]],
                            )
                            nc.sync.dma_start(out=rb[par * DH:(par + 1) * DH, :],
                                              in_=bcast)
                        nc.vector.tensor_mul(
                            ot_sb[pair][:, ib * IB:(ib + 1) * IB],
                            ot_sb[pair][:, ib * IB:(ib + 1) * IB],
                            rb,
                        )
                    # ---- output projection for this i-block ----
                    for dt4 in range(NDT):
                        yp = ps2.tile([P, IB], F32, tag="pv", bufs=2)
                        for pair in range(2):
                            nc.tensor.matmul(
                                yp,
                                lhsT=wo_sb[pair][:, dt4 * P:(dt4 + 1) * P],
                                rhs=ot_sb[pair][:, ib * IB:(ib + 1) * IB],
                                start=(pair == 0), stop=(pair == 1),
                            )
                        yt_t = sb.tile([P, IB], F32, tag="yt", bufs=3)
                        nc.vector.tensor_scalar_add(yt_t, yp, bias_sb[dt4])
                        nc.sync.dma_start(
                            out=yT[dt4 * P:(dt4 + 1) * P, ib * IB:(ib + 1) * IB],
                            in_=yt_t,
                        )

    nc.finalize()
    return nc


_nc_cache = {}


def _get_program(n_reps):
    if n_reps not in _nc_cache:
        _nc_cache[n_reps] = build_program(n_reps)
    return _nc_cache[n_reps]


def make_in_maps(x, w_qkv, w_out, b_out):
    x = np.asarray(x, np.float32)
    w_qkv = np.asarray(w_qkv, np.float32)
    w_out = np.asarray(w_out, np.float32)
    b_out = np.asarray(b_out, np.float32)
    in_maps = []
    for core in range(N_CORES):
        b, hg = core // 2, core % 2
        s = 256 * hg
        wq = w_qkv[s:s + 256]
        wk = w_qkv[512 + s:512 + s + 256]
        wv_ = w_qkv[1024 + s:1024 + s + 256]
        in_maps.append({
            "xT": np.ascontiguousarray(x[b].T).astype(BF16NP),
            "wqk": np.ascontiguousarray(np.concatenate([wq, wk], 0).T).astype(BF16NP),
            "wv": np.ascontiguousarray(wv_.T).astype(BF16NP),
            "wo": np.ascontiguousarray(w_out[:, s:s + 256].T).astype(BF16NP),
            "bias": np.ascontiguousarray((b_out / 2).reshape(D, 1)),
        })
    return in_maps


def kernel(x, w_qkv, w_out, b_out):
    nc = _get_program(N_REPS)
    in_maps = make_in_maps(x, w_qkv, w_out, b_out)
    res = run_bass_kernel_spmd(nc, in_maps, list(range(N_CORES)))
    out = np.empty((B, N, D), np.float32)
    for b in range(B):
        out[b] = (res.results[2 * b]["yT"] + res.results[2 * b + 1]["yT"]).T
    return out


if __name__ == "__main__":
    nc = build_program(1)
    print("built OK; instructions:",
          sum(len(blk.instructions) for f in nc.m.functions for blk in f.blocks))


# revision 12
# speedup vs baseline: 1.0279x; 1.0279x over previous
"""Multi-head attention (B=4, N=2048, D=512, H=8, Dh=64) on 8 trn2 cores.

Sharding: core c handles batch b = c//2 and head-group hg = c%2 (4 heads =
2 pairs).  Each core computes its batch's attention output for its 4 heads
plus the partial output projection; the host sums the two head-group
partials per batch.

v3: every matmul in the attention steady state runs in 64-row PE-tiling
mode (tiles T0/T8) so the array never pays a mode-switch drain (~105ns),
and row-tile pairs stream concurrently (2x) where the contraction is 64:
 - Scores S^T per pair: T0 computes the even head, T8 the odd head,
   concurrently, into the two banks of one [128, 1024] PSUM tile.
 - exp on the scalar engine per jt ([128, 1024], ~1020ns each).
 - PV splits its 128-contraction into T0/T8 j-halves (concurrent), with a
   [V|ones] 65-column stationary; the two half-results and half-denominators
   are combined by single fused DVE adds on PSUM eviction.
 - Denominators are batch-reciprocaled ([16,128] DVE op per i-block) and
   broadcast back via a DRAM bounce; normalization and the output
   projection for block ib run interleaved under block ib+1's attention.
All PE operands bf16 (f32 PSUM accumulation); QKV projection runs in full
128-row mode as a separate phase.
"""

import sys

for p in ("/opt/trn_rl_repo", "/root/.axon_site/_ro/trn_rl_repo"):
    if p not in sys.path:
        sys.path.insert(0, p)

from contextlib import ExitStack

import numpy as np
import ml_dtypes

import concourse.bass as bass
import concourse.mybir as mybir
import concourse.tile as tile
from concourse import bacc
from concourse.bass_utils import run_bass_kernel_spmd

F32 = mybir.dt.float32
BF16 = mybir.dt.bfloat16
AF = mybir.ActivationFunctionType
BF16NP = ml_dtypes.bfloat16

N_CORES = 8
B, N, D = 4, 2048, 512
HEADS = 8
DH = 64
SCALE = DH**-0.5
HPC = 4  # heads per core (2 pairs)
P = 128
NDT = D // P  # 4 d-tiles
NJT = N // P  # 16 j-tiles
IB = 512  # i-block
NIB = N // IB  # 4 i-blocks

N_REPS = 1  # replications of the whole body inside one NEFF (for timing)


def build_program(n_reps: int = N_REPS):
    nc = bacc.Bacc("TRN2", target_bir_lowering=False, debug=False,
                   num_devices=N_CORES)
    xT = nc.dram_tensor("xT", [D, N], BF16, kind="ExternalInput").ap()
    wqk = nc.dram_tensor("wqk", [D, 2 * HPC * DH], BF16, kind="ExternalInput").ap()
    wv = nc.dram_tensor("wv", [D, HPC * DH], BF16, kind="ExternalInput").ap()
    wo = nc.dram_tensor("wo", [HPC * DH, D], BF16, kind="ExternalInput").ap()
    bias = nc.dram_tensor("bias", [D, 1], F32, kind="ExternalInput").ap()
    yT = nc.dram_tensor("yT", [D, N], F32, kind="ExternalOutput").ap()
    # DRAM scratch: denominator j-halves per i-block (cols 0:HPC*IB = half0,
    # HPC*IB: = half1), and reciprocals laid out [ib, (2*pair+par)*IB + i].
    den_d = nc.dram_tensor("den_d", [NIB, 2 * HPC * IB], F32).ap()
    rden_d = nc.dram_tensor("rden_d", [NIB, HPC * IB], BF16).ap()

    with tile.TileContext(nc) as tc, ExitStack() as ctx:
        sb = ctx.enter_context(tc.tile_pool(name="sb", bufs=1))
        if n_reps > 1:
            ctx.enter_context(tc.For_i(0, n_reps, 1))

        for _rep in range(1):
            # preload the exp activation table while the input DMAs run
            warm = sb.tile([1, 16], F32, tag="warm", bufs=1)
            nc.vector.memset(warm, 0.0)
            nc.scalar.activation(warm, warm, AF.Exp, scale=1.0)

            # ---------------- phase 1: load + QKV projection ----------------
            wqk_sb = []
            wv_sb = []
            bias_sb = []
            for dt in range(NDT):
                t = sb.tile([P, 2 * HPC * DH], BF16, tag="wqk", bufs=NDT)
                nc.sync.dma_start(out=t, in_=wqk[dt * P:(dt + 1) * P, :])
                wqk_sb.append(t)
                t = sb.tile([P, HPC * DH], BF16, tag="wv", bufs=NDT)
                nc.sync.dma_start(out=t, in_=wv[dt * P:(dt + 1) * P, :])
                wv_sb.append(t)
                t = sb.tile([P, 1], F32, tag="bias", bufs=NDT)
                nc.sync.dma_start(out=t, in_=bias[dt * P:(dt + 1) * P, :])
                bias_sb.append(t)
            wo_sb = []
            for h in range(HPC):
                t = sb.tile([DH, D], BF16, tag="wo", bufs=HPC)
                nc.sync.dma_start(out=t, in_=wo[h * DH:(h + 1) * DH, :])
                wo_sb.append(t)

            xt_sb = []
            for dt in range(NDT):
                t = sb.tile([P, N], BF16, tag="xt", bufs=NDT)
                nc.sync.dma_start(out=t, in_=xT[dt * P:(dt + 1) * P, :])
                xt_sb.append(t)

            # Q^T/K^T tiles [128, N]; rows 0:64 even head of pair, 64:128 odd.
            # et: 0 = Q pair0, 1 = Q pair1, 2 = K pair0, 3 = K pair1
            qkt_sb = []
            with tc.tile_pool(name="ps1", bufs=1, space="PSUM") as ps1:
                for et in range(4):
                    t = sb.tile([P, N], BF16, tag="qkt", bufs=4)
                    qkt_sb.append(t)
                    for nb in range(NIB):
                        pq = ps1.tile([P, IB], F32, tag="qk", bufs=4)
                        for dt in range(NDT):
                            nc.tensor.matmul(
                                pq,
                                lhsT=wqk_sb[dt][:, et * P:(et + 1) * P],
                                rhs=xt_sb[dt][:, nb * IB:(nb + 1) * IB],
                                start=(dt == 0), stop=(dt == NDT - 1),
                            )
                        nc.vector.tensor_copy(t[:, nb * IB:(nb + 1) * IB], pq)

                # V natural [n, e] with a ones column per head: [128, 4*65]
                v_sb = []
                for nt in range(NJT):
                    t = sb.tile([P, HPC * (DH + 1)], BF16, tag="v", bufs=NJT)
                    v_sb.append(t)
                    pv = ps1.tile([P, HPC * DH], F32, tag="v", bufs=2)
                    for dt in range(NDT):
                        nc.tensor.matmul(
                            pv,
                            lhsT=xt_sb[dt][:, nt * P:(nt + 1) * P],
                            rhs=wv_sb[dt],
                            start=(dt == 0), stop=(dt == NDT - 1),
                        )
                    nc.vector.tensor_copy(
                        t.rearrange("p (h c) -> p h c", c=DH + 1)[:, :, 0:DH],
                        pv.rearrange("p (h c) -> p h c", c=DH),
                    )
                    nc.vector.memset(
                        t.rearrange("p (h c) -> p h c", c=DH + 1)[:, :, DH:DH + 1],
                        1.0,
                    )

            # ---------------- phase 2: attention (all 64-row mode) ----------
            # O^T per head [64, N] (partitions 0:64), unnormalized.
            ot_sb = [sb.tile([DH, N], BF16, tag="ot", bufs=HPC, name=f"ot{_h}")
                     for _h in range(HPC)]

            def proj_block(ps, ibp):
                """normalize + output projection for i-block ibp"""
                for pair in range(2):
                    for par in range(2):
                        h = 2 * pair + par
                        rb = sb.tile([DH, IB], BF16, tag="rb", bufs=4)
                        src = rden_d[ibp, h * IB:(h + 1) * IB]
                        bcast = bass.AP(
                            tensor=src.tensor, offset=src.offset,
                            ap=[[0, DH]] + [list(d) for d in src.ap[-1:]],
                        )
                        nc.sync.dma_start(out=rb, in_=bcast)
                        nc.vector.tensor_mul(
                            ot_sb[h][:, ibp * IB:(ibp + 1) * IB],
                            ot_sb[h][:, ibp * IB:(ibp + 1) * IB],
                            rb,
                        )
                for dt4 in range(NDT):
                    yp = ps.tile([P, IB], F32, tag="pv", bufs=4)
                    for h in range(HPC):
                        nc.tensor.matmul(
                            yp,
                            lhsT=wo_sb[h][:, dt4 * P:(dt4 + 1) * P],
                            rhs=ot_sb[h][:, ibp * IB:(ibp + 1) * IB],
                            start=(h == 0), stop=(h == HPC - 1),
                        )
                    yt_t = sb.tile([P, IB], F32, tag="yt", bufs=3)
                    nc.vector.tensor_scalar_add(yt_t, yp, bias_sb[dt4])
                    nc.sync.dma_start(
                        out=yT[dt4 * P:(dt4 + 1) * P, ibp * IB:(ibp + 1) * IB],
                        in_=yt_t,
                    )

            with tc.tile_pool(name="ps2", bufs=1, space="PSUM") as ps2:
                for ib in range(NIB):
                    den_c = sb.tile([DH + 1, 2 * HPC * IB], F32, tag="den",
                                    bufs=2)
                    for pair in range(2):
                        he, ho = 2 * pair, 2 * pair + 1
                        # 4 pv banks: a0/a1 = even-head j-halves, b0/b1 = odd
                        pv_t = [ps2.tile([P, IB], F32, tag="pv", bufs=4,
                                         name=f"pv{_k}") for _k in range(4)]
                        for jt in range(NJT):
                            su = ps2.tile([P, 2 * IB], F32, tag="s", bufs=2)
                            for par in range(2):
                                lo, hi = par * DH, (par + 1) * DH
                                nc.tensor.matmul(
                                    su[:, par * IB:(par + 1) * IB],
                                    lhsT=qkt_sb[2 + pair][lo:hi,
                                                          jt * P:(jt + 1) * P],
                                    rhs=qkt_sb[pair][lo:hi,
                                                     ib * IB:(ib + 1) * IB],
                                    start=True, stop=True,
                                )
                            es = sb.tile([P, 2 * IB], BF16, tag="big", bufs=8)
                            nc.scalar.activation(es, su, AF.Exp, scale=SCALE)
                            for par in range(2):
                                h = 2 * pair + par
                                for jh in range(2):
                                    nc.tensor.matmul(
                                        pv_t[2 * par + jh][0:DH + 1, :],
                                        lhsT=v_sb[jt][jh * DH:(jh + 1) * DH,
                                                      h * (DH + 1):
                                                      (h + 1) * (DH + 1)],
                                        rhs=es[jh * DH:(jh + 1) * DH,
                                               par * IB:(par + 1) * IB],
                                        start=(jt == 0), stop=(jt == NJT - 1),
                                    )
                        # eviction: O = half0 + half1 (copy then add; a
                        # tensor_tensor may read at most one PSUM operand),
                        # den halves collected separately on partition 64.
                        for par in range(2):
                            h = 2 * pair + par
                            a, b = pv_t[2 * par], pv_t[2 * par + 1]
                            osl = ot_sb[h][:, ib * IB:(ib + 1) * IB]
                            nc.vector.tensor_copy(osl, a[0:DH, :])
                            nc.vector.tensor_add(osl, osl, b[0:DH, :])
                            nc.vector.tensor_copy(
                                den_c[DH:DH + 1, h * IB:(h + 1) * IB],
                                a[DH:DH + 1, :],
                            )
                            nc.vector.tensor_copy(
                                den_c[DH:DH + 1, (HPC + h) * IB:
                                      (HPC + h + 1) * IB],
                                b[DH:DH + 1, :],
                            )
                    # denominator reciprocal chain for this i-block
                    nc.sync.dma_start(out=den_d[ib:ib + 1, :],
                                      in_=den_c[DH:DH + 1, :])
                    rc2a = sb.tile([16, P], F32, tag="rc2a", bufs=2)
                    nc.sync.dma_start(
                        out=rc2a,
                        in_=den_d[ib:ib + 1, 0:HPC * IB].rearrange(
                            "a (q p) -> (a q) p", p=P))
                    rc2b = sb.tile([16, P], F32, tag="rc2b", bufs=2)
                    nc.sync.dma_start(
                        out=rc2b,
                        in_=den_d[ib:ib + 1, HPC * IB:].rearrange(
                            "a (q p) -> (a q) p", p=P))
                    rc = sb.tile([16, P], F32, tag="rc", bufs=2)
                    nc.vector.tensor_add(rc, rc2a, rc2b)
                    nc.vector.reciprocal(rc, rc)
                    rcb = sb.tile([16, P], BF16, tag="rcb", bufs=2)
                    nc.vector.tensor_copy(rcb, rc)
                    nc.sync.dma_start(
                        out=rden_d[ib:ib + 1, :].rearrange(
                            "a (q p) -> (a q) p", p=P),
                        in_=rcb)
                    if ib > 0:
                        proj_block(ps2, ib - 1)
                proj_block(ps2, NIB - 1)

    nc.finalize()
    return nc


_nc_cache = {}


def _get_program(n_reps):
    if n_reps not in _nc_cache:
        _nc_cache[n_reps] = build_program(n_reps)
    return _nc_cache[n_reps]


def make_in_maps(x, w_qkv, w_out, b_out):
    x = np.asarray(x, np.float32)
    w_qkv = np.asarray(w_qkv, np.float32)
    w_out = np.asarray(w_out, np.float32)
    b_out = np.asarray(b_out, np.float32)
    in_maps = []
    for core in range(N_CORES):
        b, hg = core // 2, core % 2
        s = 256 * hg
        wq = w_qkv[s:s + 256]
        wk = w_qkv[512 + s:512 + s + 256]
        wv_ = w_qkv[1024 + s:1024 + s + 256]
        in_maps.append({
            "xT": np.ascontiguousarray(x[b].T).astype(BF16NP),
            "wqk": np.ascontiguousarray(np.concatenate([wq, wk], 0).T).astype(BF16NP),
            "wv": np.ascontiguousarray(wv_.T).astype(BF16NP),
            "wo": np.ascontiguousarray(w_out[:, s:s + 256].T).astype(BF16NP),
            "bias": np.ascontiguousarray((b_out / 2).reshape(D, 1)),
        })
    return in_maps


def kernel(x, w_qkv, w_out, b_out):
    nc = _get_program(N_REPS)
    in_maps = make_in_maps(x, w_qkv, w_out, b_out)
    res = run_bass_kernel_spmd(nc, in_maps, list(range(N_CORES)))
    out = np.empty((B, N, D), np.float32)
    for b in range(B):
        out[b] = (res.results[2 * b]["yT"] + res.results[2 * b + 1]["yT"]).T
    return out


if __name__ == "__main__":
    nc = build_program(1)
    print("built OK; instructions:",
          sum(len(blk.instructions) for f in nc.m.functions for blk in f.blocks))


# revision 16
# speedup vs baseline: 1.7056x; 1.6593x over previous
"""Multi-head attention (B=4, N=2048, D=512, H=8, Dh=64) on 8 trn2 cores.

Sharding: core c handles batch b = c//2 and head-group hg = c%2 (4 heads =
2 pairs).  Each core computes its batch's attention output for its 4 heads
plus the partial output projection; the host sums the two head-group
partials per batch.

v3: every matmul in the attention steady state runs in 64-row PE-tiling
mode (tiles T0/T8) so the array never pays a mode-switch drain (~105ns),
and row-tile pairs stream concurrently (2x) where the contraction is 64:
 - Scores S^T per pair: T0 computes the even head, T8 the odd head,
   concurrently, into the two banks of one [128, 1024] PSUM tile.
 - exp on the scalar engine per jt ([128, 1024], ~1020ns each).
 - PV splits its 128-contraction into T0/T8 j-halves (concurrent), with a
   [V|ones] 65-column stationary; the two half-results and half-denominators
   are combined by single fused DVE adds on PSUM eviction.
 - Denominators are batch-reciprocaled ([16,128] DVE op per i-block) and
   broadcast back via a DRAM bounce; normalization and the output
   projection for block ib run interleaved under block ib+1's attention.
All PE operands bf16 (f32 PSUM accumulation); QKV projection runs in full
128-row mode as a separate phase.
"""

import sys

for p in ("/opt/trn_rl_repo", "/root/.axon_site/_ro/trn_rl_repo"):
    if p not in sys.path:
        sys.path.insert(0, p)

from contextlib import ExitStack

import numpy as np
import ml_dtypes

import concourse.bass as bass
import concourse.mybir as mybir
import concourse.tile as tile
from concourse import bacc
from concourse.bass_utils import run_bass_kernel_spmd

F32 = mybir.dt.float32
BF16 = mybir.dt.bfloat16
AF = mybir.ActivationFunctionType
BF16NP = ml_dtypes.bfloat16

N_CORES = 8
B, N, D = 4, 2048, 512
HEADS = 8
DH = 64
SCALE = DH**-0.5
HPC = 4  # heads per core (2 pairs)
P = 128
NDT = D // P  # 4 d-tiles
NJT = N // P  # 16 j-tiles
IB = 512  # i-block
NIB = N // IB  # 4 i-blocks

N_REPS = 1  # replications of the whole body inside one NEFF (for timing)


def build_program(n_reps: int = N_REPS):
    nc = bacc.Bacc("TRN2", target_bir_lowering=False, debug=False,
                   num_devices=N_CORES)
    xT = nc.dram_tensor("xT", [D, N], BF16, kind="ExternalInput").ap()
    wqk = nc.dram_tensor("wqk", [D, 2 * HPC * DH], BF16, kind="ExternalInput").ap()
    wv = nc.dram_tensor("wv", [D, HPC * DH], BF16, kind="ExternalInput").ap()
    wo = nc.dram_tensor("wo", [HPC * DH, D], BF16, kind="ExternalInput").ap()
    bias = nc.dram_tensor("bias", [D, 1], F32, kind="ExternalInput").ap()
    yT = nc.dram_tensor("yT", [D, N], F32, kind="ExternalOutput").ap()
    # DRAM scratch: denominators and reciprocals, [ib, h*IB + i], bf16.
    den_d = nc.dram_tensor("den_d", [NIB, HPC * IB], BF16).ap()
    rden_d = nc.dram_tensor("rden_d", [NIB, HPC * IB], BF16).ap()

    with tile.TileContext(nc) as tc, ExitStack() as ctx:
        sb = ctx.enter_context(tc.tile_pool(name="sb", bufs=1))
        if n_reps > 1:
            ctx.enter_context(tc.For_i(0, n_reps, 1))

        for _rep in range(1):
            # preload the exp activation table while the input DMAs run
            warm = sb.tile([1, 16], F32, tag="warm", bufs=1)
            nc.vector.memset(warm, 0.0)
            nc.scalar.activation(warm, warm, AF.Exp, scale=1.0)

            # ---------------- phase 1: load + QKV projection ----------------
            wqk_sb = []
            wv_sb = []
            bias_sb = []
            for dt in range(NDT):
                t = sb.tile([P, 2 * HPC * DH], BF16, tag="wqk", bufs=NDT)
                nc.sync.dma_start(out=t, in_=wqk[dt * P:(dt + 1) * P, :])
                wqk_sb.append(t)
                t = sb.tile([P, HPC * DH], BF16, tag="wv", bufs=NDT)
                nc.sync.dma_start(out=t, in_=wv[dt * P:(dt + 1) * P, :])
                wv_sb.append(t)
                t = sb.tile([P, 1], F32, tag="bias", bufs=NDT)
                nc.sync.dma_start(out=t, in_=bias[dt * P:(dt + 1) * P, :])
                bias_sb.append(t)
            wo_sb = []
            for h in range(HPC):
                t = sb.tile([DH, D], BF16, tag="wo", bufs=HPC)
                nc.sync.dma_start(out=t, in_=wo[h * DH:(h + 1) * DH, :])
                wo_sb.append(t)

            xt_sb = []
            for dt in range(NDT):
                t = sb.tile([P, N], BF16, tag="xt", bufs=NDT)
                nc.sync.dma_start(out=t, in_=xT[dt * P:(dt + 1) * P, :])
                xt_sb.append(t)

            # Q^T/K^T tiles [128, N]; rows 0:64 even head of pair, 64:128 odd.
            # et: 0 = Q pair0, 1 = Q pair1, 2 = K pair0, 3 = K pair1
            qkt_sb = []
            with tc.tile_pool(name="ps1", bufs=1, space="PSUM") as ps1:
                for et in range(4):
                    t = sb.tile([P, N], BF16, tag="qkt", bufs=4)
                    qkt_sb.append(t)
                    for nb in range(NIB):
                        pq = ps1.tile([P, IB], F32, tag="qk", bufs=4)
                        for dt in range(NDT):
                            nc.tensor.matmul(
                                pq,
                                lhsT=wqk_sb[dt][:, et * P:(et + 1) * P],
                                rhs=xt_sb[dt][:, nb * IB:(nb + 1) * IB],
                                start=(dt == 0), stop=(dt == NDT - 1),
                            )
                        nc.vector.tensor_copy(t[:, nb * IB:(nb + 1) * IB], pq)

                # V natural [n, e] with a ones column per head: [128, 4*65]
                v_sb = []
                for nt in range(NJT):
                    t = sb.tile([P, HPC * (DH + 1)], BF16, tag="v", bufs=NJT)
                    v_sb.append(t)
                    pv = ps1.tile([P, HPC * DH], F32, tag="v", bufs=2)
                    for dt in range(NDT):
                        nc.tensor.matmul(
                            pv,
                            lhsT=xt_sb[dt][:, nt * P:(nt + 1) * P],
                            rhs=wv_sb[dt],
                            start=(dt == 0), stop=(dt == NDT - 1),
                        )
                    nc.vector.tensor_copy(
                        t.rearrange("p (h c) -> p h c", c=DH + 1)[:, :, 0:DH],
                        pv.rearrange("p (h c) -> p h c", c=DH),
                    )
                    nc.vector.memset(
                        t.rearrange("p (h c) -> p h c", c=DH + 1)[:, :, DH:DH + 1],
                        1.0,
                    )

            # ---------------- phase 2: attention (all 64-row mode) ----------
            # O^T per head [65, N]: rows 0:64 = O (partitions 0:64), row 64 =
            # softmax denominator (rides along in the PSUM eviction).
            ot_sb = [sb.tile([DH + 1, N], BF16, tag="ot", bufs=HPC,
                             name=f"ot{_h}") for _h in range(HPC)]

            units = [(ib, pair, jt) for ib in range(NIB)
                     for pair in range(2) for jt in range(NJT)]
            su_t = {}

            with tc.tile_pool(name="ps2", bufs=1, space="PSUM") as ps2:
                def emit_scores(k):
                    ib, pair, jt = units[k]
                    su = ps2.tile([P, 2 * IB], F32, tag="s", bufs=2,
                                  name=f"su{k}")
                    for par in range(2):
                        lo, hi = par * DH, (par + 1) * DH
                        nc.tensor.matmul(
                            su[:, par * IB:(par + 1) * IB],
                            lhsT=qkt_sb[2 + pair][lo:hi, jt * P:(jt + 1) * P],
                            rhs=qkt_sb[pair][lo:hi, ib * IB:(ib + 1) * IB],
                            start=True, stop=True,
                        )
                    su_t[k] = su

                def emit_proj_half(ibp, half):
                    # output projection for 2 of the 4 d-tiles of block ibp,
                    # borrowing a su-tag PSUM slot ([128, 1024] = 2 banks).
                    yp2 = ps2.tile([P, 2 * IB], F32, tag="s", bufs=2,
                                   name=f"yp{ibp}_{half}")
                    yt_t = sb.tile([P, 2 * IB], F32, tag="yt", bufs=2)
                    for i in range(2):
                        dt4 = 2 * half + i
                        for h in range(HPC):
                            nc.tensor.matmul(
                                yp2[:, i * IB:(i + 1) * IB],
                                lhsT=wo_sb[h][:, dt4 * P:(dt4 + 1) * P],
                                rhs=ot_sb[h][0:DH, ibp * IB:(ibp + 1) * IB],
                                start=(h == 0), stop=(h == HPC - 1),
                            )
                        nc.vector.tensor_scalar_add(
                            yt_t[:, i * IB:(i + 1) * IB],
                            yp2[:, i * IB:(i + 1) * IB], bias_sb[dt4])
                        nc.sync.dma_start(
                            out=yT[dt4 * P:(dt4 + 1) * P,
                                   ibp * IB:(ibp + 1) * IB],
                            in_=yt_t[:, i * IB:(i + 1) * IB],
                        )

                def emit_chain(ib):
                    # denominator reciprocal + normalization for block ib
                    for h in range(HPC):
                        nc.sync.dma_start(
                            out=den_d[ib:ib + 1, h * IB:(h + 1) * IB],
                            in_=ot_sb[h][DH:DH + 1, ib * IB:(ib + 1) * IB])
                    rc2 = sb.tile([16, P], BF16, tag="rc2", bufs=2)
                    nc.sync.dma_start(
                        out=rc2,
                        in_=den_d[ib:ib + 1, :].rearrange(
                            "a (q p) -> (a q) p", p=P))
                    with nc.allow_low_precision(
                            reason="bf16 reciprocal of softmax denom, 0.4% ok"):
                        nc.vector.reciprocal(rc2, rc2)
                    nc.sync.dma_start(
                        out=rden_d[ib:ib + 1, :].rearrange(
                            "a (q p) -> (a q) p", p=P),
                        in_=rc2)
                    for h in range(HPC):
                        rb = sb.tile([DH, IB], BF16, tag="rb", bufs=4)
                        src = rden_d[ib, h * IB:(h + 1) * IB]
                        bcast = bass.AP(
                            tensor=src.tensor, offset=src.offset,
                            ap=[[0, DH]] + [list(d) for d in src.ap[-1:]],
                        )
                        nc.sync.dma_start(out=rb, in_=bcast)
                        nc.gpsimd.tensor_mul(
                            ot_sb[h][0:DH, ib * IB:(ib + 1) * IB],
                            ot_sb[h][0:DH, ib * IB:(ib + 1) * IB],
                            rb,
                        )

                emit_scores(0)
                pv_t = None
                for k, (ib, pair, jt) in enumerate(units):
                    if jt == 0:
                        pv_t = [ps2.tile([P, IB], F32, tag="pv", bufs=4,
                                         name=f"pv{ib}_{pair}_{_k}")
                                for _k in range(4)]
                    if k + 1 < len(units):
                        emit_scores(k + 1)
                    es = sb.tile([P, 2 * IB], BF16, tag="big", bufs=8)
                    nc.scalar.activation(es, su_t.pop(k), AF.Exp, scale=SCALE)
                    for par in range(2):
                        h = 2 * pair + par
                        for jh in range(2):
                            nc.tensor.matmul(
                                pv_t[2 * par + jh][0:DH + 1, :],
                                lhsT=v_sb[jt][jh * DH:(jh + 1) * DH,
                                              h * (DH + 1):(h + 1) * (DH + 1)],
                                rhs=es[jh * DH:(jh + 1) * DH,
                                       par * IB:(par + 1) * IB],
                                start=(jt == 0), stop=(jt == NJT - 1),
                            )
                    if pair == 0 and ib >= 1 and jt in (6, 11):
                        emit_proj_half(ib - 1, 0 if jt == 6 else 1)
                    if jt == NJT - 1:
                        # evict this block's PV accumulators (O and den rows)
                        for par in range(2):
                            h = 2 * pair + par
                            a, b = pv_t[2 * par], pv_t[2 * par + 1]
                            osl = ot_sb[h][:, ib * IB:(ib + 1) * IB]
                            nc.vector.tensor_copy(osl, a[0:DH + 1, :])
                            with nc.allow_low_precision(
                                    reason="bf16 j-half merge, 0.4% ok"):
                                nc.vector.tensor_add(osl, osl, b[0:DH + 1, :])
                        if pair == 1:
                            emit_chain(ib)
                emit_proj_half(NIB - 1, 0)
                emit_proj_half(NIB - 1, 1)

    nc.finalize()
    return nc


_nc_cache = {}


def _get_program(n_reps):
    if n_reps not in _nc_cache:
        _nc_cache[n_reps] = build_program(n_reps)
    return _nc_cache[n_reps]


def make_in_maps(x, w_qkv, w_out, b_out):
    x = np.asarray(x, np.float32)
    w_qkv = np.asarray(w_qkv, np.float32)
    w_out = np.asarray(w_out, np.float32)
    b_out = np.asarray(b_out, np.float32)
    in_maps = []
    for core in range(N_CORES):
        b, hg = core // 2, core % 2
        s = 256 * hg
        wq = w_qkv[s:s + 256]
        wk = w_qkv[512 + s:512 + s + 256]
        wv_ = w_qkv[1024 + s:1024 + s + 256]
        in_maps.append({
            "xT": np.ascontiguousarray(x[b].T).astype(BF16NP),
            "wqk": np.ascontiguousarray(np.concatenate([wq, wk], 0).T).astype(BF16NP),
            "wv": np.ascontiguousarray(wv_.T).astype(BF16NP),
            "wo": np.ascontiguousarray(w_out[:, s:s + 256].T).astype(BF16NP),
            "bias": np.ascontiguousarray((b_out / 2).reshape(D, 1)),
        })
    return in_maps


def kernel(x, w_qkv, w_out, b_out):
    nc = _get_program(N_REPS)
    in_maps = make_in_maps(x, w_qkv, w_out, b_out)
    res = run_bass_kernel_spmd(nc, in_maps, list(range(N_CORES)))
    out = np.empty((B, N, D), np.float32)
    for b in range(B):
        out[b] = (res.results[2 * b]["yT"] + res.results[2 * b + 1]["yT"]).T
    return out


if __name__ == "__main__":
    nc = build_program(1)
    print("built OK; instructions:",
          sum(len(blk.instructions) for f in nc.m.functions for blk in f.blocks))
